# revision 38
# baseline (speedup 1.0000x reference)
"""DRew-GIN layer on 8 TRN2 NeuronCores.

Strategy (source-sharded, chunked ReduceScatter, channel-major partials):
  - Nodes are sharded 8 ways. Core c computes the three coef-scaled hop MLP
    tables h'_k = hop_coef[k-1] * MLP_k(emb_src_k) for its node slice only,
    node-major bf16 in local DRAM (W2/b2 are pre-scaled by hop_coef on the
    host; MLP runs bf16 in / f32 PSUM).  The self-loop MLP table for the
    slice stays resident in SBUF.
  - Edges are partitioned by SOURCE core. Each core produces a PARTIAL
    aggregate over the full (padded) destination range: edges are sorted by
    destination window (128 dest rows); per 128-edge tile we dma_gather the
    bf16 source rows (256B each, spread over 4 SWDGE queues) from the local
    h' table, build a one-hot S[e, d] = (slot[e] == d) on DVE (bf16,
    transposed batch layout so the 2x_1p fast mode applies), and
    matmul-accumulate G^T @ S into the window's PSUM tile, which is
    CHANNEL-major [ch, slot] (f32).  Flushes write bf16 channel-major
    blocks with 2KB-per-partition runs (full DMA efficiency).
  - The partial stream is laid out [chunk][dest-core][C][cols]; windows are
    processed in (chunk, dest-core, window) order, so as each chunk's
    windows finish flushing, a contiguous ReduceScatter(add) for just that
    chunk fires, overlapping the collective with later chunks' work.  The
    per-chunk finale PE-transposes back to node-major, adds the
    SBUF-resident self term, and writes the output slice.
"""

import math
import sys

sys.path.insert(0, "/opt/trn_rl_repo")

import ml_dtypes
import numpy as np

import concourse.bacc as bacc
import concourse.bass as bass
import concourse.tile as tile
from concourse import mybir
from concourse.bass_utils import run_bass_kernel_spmd

NCORES = 8
C = 128
P = 128
GBLK = 32  # gather block = 32 tiles = 4096 indices
NCHUNKS = 4  # ReduceScatter pipeline depth


def make_cfg(n_nodes, n_edges):
    assert n_nodes % NCORES == 0
    slice_ = n_nodes // NCORES
    slice_pad = ((slice_ + P - 1) // P) * P
    wps = slice_pad // P  # windows per dest slice
    cfg = dict(
        N=n_nodes,
        E=n_edges,
        SLICE=slice_,
        SLICE_PAD=slice_pad,
        TBL=3 * slice_pad,  # h' table rows per core
        DSLICE_PAD=slice_pad,
        WINDOWS=NCORES * wps,
        WPS=wps,
    )
    return cfg


# ---------------------------------------------------------------- host prep


def chunk_lens(wps, nchunks=NCHUNKS):
    """Uneven chunk split: bigger chunks first, small final chunk so the
    exposed tail (last ReduceScatter + finale) is short."""
    nchunks = min(nchunks, wps)
    base, rem = wps // nchunks, wps % nchunks
    lens = [base + (1 if i < rem else 0) for i in range(nchunks)]
    if UNEVEN and nchunks >= 3 and lens[-1] > 1:
        take = lens[-1] - max(1, lens[-1] // 2)
        lens[-1] -= take
        for i in range(take):
            lens[i % (nchunks - 1)] += 1
    return lens


def prep_edges(cfg, row, col, ew, nchunks=NCHUNKS):
    """Returns (per_core {gidx,slots}, meta).

    Destination windows are ranked in (chunk, dest-core, window-in-chunk)
    order.  The edge stream is packed tightly per (chunk, dest-core) BLOCK
    (padding only at block ends, to the max edge count over source cores);
    within a block, edges sort by (window rank, slot).  Because per-core
    window boundaries fall at different stream positions, each gather tile
    carries one matmul RECORD per window it may span on any core (the
    union range); a record's one-hot column holds a core's slots only for
    its edges of that (tile, window), so off-range cores contribute zeros.
    """
    N, SLICE, SLICE_PAD, WINDOWS, WPS = (
        cfg["N"],
        cfg["SLICE"],
        cfg["SLICE_PAD"],
        cfg["WINDOWS"],
        cfg["WPS"],
    )
    if LENS_OVERRIDE is not None and sum(LENS_OVERRIDE) == WPS:
        lens = list(LENS_OVERRIDE)
    else:
        lens = chunk_lens(WPS, nchunks)
    nchunks = len(lens)
    cumwi = np.concatenate([[0], np.cumsum(lens)])  # wi-space chunk bounds

    # rank order: (chunk, dest core, wi)
    order_w = []
    for ci in range(nchunks):
        for sc in range(NCORES):
            for wi in range(cumwi[ci], cumwi[ci + 1]):
                order_w.append(sc * WPS + wi)
    order_w = np.asarray(order_w)
    wrank = np.empty(WINDOWS, np.int64)
    wrank[order_w] = np.arange(WINDOWS)
    chunk_of_rank = np.repeat(np.arange(nchunks), np.asarray(lens) * NCORES)

    row = row.astype(np.int64)
    col = col.astype(np.int64)
    ew = ew.astype(np.int64)
    s = col // SLICE
    local = col - s * SLICE
    trow = (ew - 1) * SLICE_PAD + local
    assert trow.max() < 3 * SLICE_PAD <= 32767
    w = (row // SLICE) * WPS + (row % SLICE) // P  # dest window
    slot = (row % SLICE) % P
    r = wrank[w]

    key = s * WINDOWS + r
    order = np.argsort(key, kind="stable")
    key_s = key[order]
    counts = np.bincount(key_s, minlength=NCORES * WINDOWS).reshape(NCORES, WINDOWS)

    # blocks: consecutive rank runs of one (chunk, dest core); block bi has
    # ranks [blk_r0[bi], blk_r0[bi+1])
    blk_r0 = [0]
    for ci in range(nchunks):
        for _ in range(NCORES):
            blk_r0.append(blk_r0[-1] + lens[ci])
    nblocks = len(blk_r0) - 1

    # per-core positions: edges sorted by (core, rank); position within the
    # (core, BLOCK) group = cumulative count within the block
    group_starts = np.concatenate([[0], np.cumsum(counts.reshape(-1))])[:-1]
    pos_in_rank = np.arange(len(key_s)) - group_starts[key_s]
    core_of = key_s // WINDOWS
    rank_of = key_s % WINDOWS

    # per-core cumulative start of each rank within its block
    rank_block = np.zeros(WINDOWS, np.int64)  # rank -> block
    for bi in range(nblocks):
        rank_block[blk_r0[bi] : blk_r0[bi + 1]] = bi
    start_in_block = np.zeros((NCORES, WINDOWS), np.int64)
    for bi in range(nblocks):
        rr = slice(blk_r0[bi], blk_r0[bi + 1])
        c = counts[:, rr]
        start_in_block[:, rr] = np.cumsum(c, axis=1) - c

    # block tile counts (max over cores) and offsets
    blk_tiles = np.zeros(nblocks, np.int64)
    for bi in range(nblocks):
        tot = counts[:, blk_r0[bi] : blk_r0[bi + 1]].sum(axis=1)
        blk_tiles[bi] = max(1, int(np.ceil(tot.max() / P)))
    blk_t0 = np.concatenate([[0], np.cumsum(blk_tiles)])
    T_total = int(blk_t0[-1])
    B = (T_total + GBLK - 1) // GBLK

    # matmul records: per window, union tile range over cores (global tiles)
    u0 = np.zeros(WINDOWS, np.int64)
    u1 = np.zeros(WINDOWS, np.int64)
    for rr in range(WINDOWS):
        bi = rank_block[rr]
        st = start_in_block[:, rr]
        en = st + counts[:, rr]
        nz = counts[:, rr] > 0
        if nz.any():
            t0 = (st[nz] // P).min()
            t1 = int(np.ceil(en[nz] / P).max())
        else:
            t0 = int(st.min() // P)
            t1 = t0 + 1
        u0[rr] = blk_t0[bi] + t0
        u1[rr] = blk_t0[bi] + max(t1, t0 + 1)

    # records in WINDOW-sequential order (PSUM start=True zeroes a whole
    # 2KB bank region, so windows sharing a bank must accumulate strictly
    # sequentially); tile index regresses by at most span-1 between
    # windows, staying within the live gather blocks
    rec_stream = []
    rec_col = {}
    for bi in range(nblocks):
        for rr in range(blk_r0[bi], blk_r0[bi + 1]):
            for t in range(int(u0[rr]), int(u1[rr])):
                rec_col[(t, rr)] = len(rec_stream)
                rec_stream.append(
                    (int(t), int(rr), bool(t == u0[rr]), bool(t == u1[rr] - 1))
                )
    M_total = len(rec_stream)

    # chunk metadata
    chunks = []
    for ci in range(nchunks):
        rank1 = int(cumwi[ci + 1]) * NCORES
        tile_end = int(blk_t0[np.searchsorted(blk_r0, rank1)]) - 1
        chunks.append(
            dict(
                rank0=int(cumwi[ci]) * NCORES,
                rank1=rank1,
                tile_end=tile_end,
                rel0=int(cumwi[ci]) * P,
                rel1=int(cumwi[ci + 1]) * P,
                length=int(lens[ci]),
            )
        )

    # per-core streams
    NPAD = T_total * P
    trow_s = trow[order]
    slot_s = slot[order]
    # global stream position of each edge: block tile base + in-block pos
    pos_block = start_in_block[core_of, rank_of] + pos_in_rank
    stream_pos = blk_t0[rank_block[rank_of]] * P + pos_block
    edge_tile = stream_pos // P
    edge_lane = stream_pos % P
    edge_colkey = [rec_col[(int(t), int(rr))] for t, rr in zip(edge_tile, rank_of)]
    edge_colkey = np.asarray(edge_colkey, np.int64)

    per_core = []
    for c in range(NCORES):
        m = core_of == c
        gidx = np.zeros(NPAD, np.int16)  # dummy -> row 0
        gidx[stream_pos[m]] = trow_s[m].astype(np.int16)
        slots = np.full((M_total, P), 255, np.float32)
        slots[edge_colkey[m], edge_lane[m]] = slot_s[m].astype(np.float32)
        # wrap gidx for dma_gather: block b, idx j -> [j%16, b*256 + j//16]
        blk = np.zeros(B * GBLK * P, np.int16)
        blk[:NPAD] = gidx
        blk = blk.reshape(B, GBLK * P // 16, 16).transpose(0, 2, 1)  # [B,16,256]
        blk = np.tile(blk, (1, 8, 1))  # [B,128,256]
        gidx_in = blk.transpose(1, 0, 2).reshape(P, -1).copy()  # [128, B*256]
        slots_in = slots.T.astype(ml_dtypes.bfloat16)  # [128, M_total]
        per_core.append(dict(gidx=gidx_in, slots=slots_in))
    meta = dict(
        rec_stream=rec_stream,
        T_total=T_total,
        M_total=M_total,
        B=B,
        chunks=chunks,
    )
    return per_core, meta


# ---------------------------------------------------------------- builder


def build_kernel(cfg, meta, debug_phases=4, comm_mode="rsc"):
    assert comm_mode == "rsc"
    SLICE, SLICE_PAD, TBL, DSLICE_PAD, WINDOWS = (
        cfg["SLICE"],
        cfg["SLICE_PAD"],
        cfg["TBL"],
        cfg["DSLICE_PAD"],
        cfg["WINDOWS"],
    )
    T_total, B = meta["T_total"], meta["B"]
    M_total = meta["M_total"]
    rec_stream = meta["rec_stream"]
    chunks = meta["chunks"]
    PTOT = NCORES * DSLICE_PAD
    f32 = mybir.dt.float32
    bf16 = mybir.dt.bfloat16
    SGRP = 8          # one-hot compare batch (tiles per DVE op)
    WQ = 8            # windows per PSUM flush group
    assert WINDOWS % WQ == 0

    # rank -> (chunk, dest core, col0) for the channel-major partial layout:
    # partial = [chunk][dest core][C rows][len_c*128 cols], flattened
    rank_info = []
    chunk_base = []
    off = 0
    for ci, ch in enumerate(chunks):
        chunk_base.append(off)
        L = ch["length"] * P  # cols per (chunk, dest-core) block
        for rr in range(ch["rank0"], ch["rank1"]):
            rl = rr - ch["rank0"]
            s_dst, wi_l = rl // ch["length"], rl % ch["length"]
            rank_info.append(
                dict(chunk=ci, block_base=off + s_dst * C * L, col0=wi_l * P, L=L)
            )
        off += NCORES * C * L
    assert off == PTOT * C

    nc = bacc.Bacc(num_swdge_queues=4)

    def param(name, shape, dt=f32):
        return nc.declare_dram_parameter(name, list(shape), dt, isOutput=False)

    embp = [param(n, [P, SLICE_PAD], bf16) for n in ("embA", "embB", "embC")]
    w1 = [param(f"w1_{i}", [P, P], bf16) for i in range(4)]  # rel, hh0, hh1, loop
    b1 = [param(f"b1_{i}", [P, 1]) for i in range(4)]
    # W2/b2 pre-scaled by hop_coef on the host
    w2s = [param(f"w2s_{i}", [P, P], bf16) for i in range(4)]
    b2s = [param(f"b2s_{i}", [P, P]) for i in range(4)]  # row-broadcast
    iota_p = param("iota", [P, P], bf16)
    ident_p = param("ident", [P, P], bf16)
    gidx_p = param("gidx", [P, B * 256], mybir.dt.int16)
    slots_p = param("slots", [P, M_total], bf16)
    tok_p = param("tok", [1, 1])
    out_ext = nc.declare_dram_parameter("out", [SLICE, C], f32, isOutput=True)
    tok_out = nc.declare_dram_parameter("tok_out", [1, 1], f32, isOutput=True)

    h_dram = nc.dram_tensor("h_tbl", [TBL, C], bf16)
    partial = nc.dram_tensor("partial", [PTOT * C], bf16)  # channel-major blocks
    rs_out = nc.dram_tensor("rs_out", [DSLICE_PAD * C], bf16)  # ch-major blocks

    def batched_rows_ap(handle, r0, nsub):
        # [p, s, ch] view of rows [r0, r0 + nsub*128) of a [rows, C] tensor
        return bass.AP(handle, r0 * C, [[C, P], [P * C, nsub], [1, C]])

    with tile.TileContext(nc) as tc:
        with (
            tc.tile_pool(name="resident", bufs=1) as rpool,
            tc.tile_pool(name="hid", bufs=3) as hpool,
            tc.tile_pool(name="hstage", bufs=4) as opool,
            tc.tile_pool(name="gather", bufs=6) as gpool,
            tc.tile_pool(name="onehot", bufs=3) as spool,
            tc.tile_pool(name="flush", bufs=4) as fpool,
            tc.tile_pool(name="fin", bufs=2) as finpool,
            tc.tile_pool(name="psA", bufs=2, space="PSUM") as psA,
            tc.tile_pool(name="psB", bufs=2, space="PSUM") as psB,
            tc.tile_pool(name="win", bufs=2, space="PSUM") as wpool,
        ):
            # ---- phase 0: resident loads
            w1_sb = [rpool.tile([P, P], bf16, tag=f"w1_{i}", name=f"w1sb{i}") for i in range(4)]
            b1_sb = [rpool.tile([P, 1], f32, tag=f"b1_{i}", name=f"b1sb{i}") for i in range(4)]
            w2s_sb = [rpool.tile([P, P], bf16, tag=f"w2s_{i}", name=f"w2ssb{i}") for i in range(4)]
            b2s_sb = [rpool.tile([P, P], f32, tag=f"b2s_{i}", name=f"b2ssb{i}") for i in range(4)]
            emb_sb = [rpool.tile([P, SLICE_PAD], bf16, tag=f"emb_{i}", name=f"embsb{i}") for i in range(3)]
            iota_sb = rpool.tile([P, P], bf16, tag="iota")
            ident_sb = rpool.tile([P, P], bf16, tag="ident")
            iota_rep = rpool.tile([P, P, SGRP], bf16, tag="iotarep", name="iotarep")
            gidx_sb = rpool.tile([P, B * 256], mybir.dt.int16, tag="gidx")
            slots_sb = rpool.tile([P, M_total], bf16, tag="slots")

            # embeddings first on SP (they gate the first MLP matmul and the
            # h_tbl writes the first gather waits on); weights and one-hot
            # metadata go through the Activation engine's HWDGE queue, which
            # is idle in the head, so SP's issue queue stays clear
            for i in range(3):
                nc.sync.dma_start(out=emb_sb[i][:], in_=embp[i][:, :])
            for i in range(4):
                nc.scalar.dma_start(out=w1_sb[i][:], in_=w1[i][:, :])
                nc.scalar.dma_start(out=b1_sb[i][:], in_=b1[i][:, :])
                nc.scalar.dma_start(out=w2s_sb[i][:], in_=w2s[i][:, :])
                nc.scalar.dma_start(out=b2s_sb[i][:], in_=b2s[i][:, :])
            nc.scalar.dma_start(out=iota_sb[:], in_=iota_p[:, :])
            nc.scalar.dma_start(out=ident_sb[:], in_=ident_p[:, :])
            # slots loads early: the DVE one-hot prebuilds consume it during
            # the table phase's DVE slack
            nc.scalar.dma_start(out=slots_sb[:], in_=slots_p[:, :])
            # iota replicated along the group dim: real packed tile so the
            # one-hot is_equal qualifies for the DVE 2x_1p fast mode
            nc.vector.tensor_copy(
                iota_rep[:], iota_sb[:].unsqueeze(2).broadcast_to([P, P, SGRP])
            )

            # ---- phase 1: MLP tables (hops 1..3 from embA/B/C; self from embA)
            # self table stays resident in SBUF (read directly by the finale)
            n_sub_tot = SLICE_PAD // P
            self_sb = rpool.tile([P, n_sub_tot, P], f32, tag="selft", name="selft")
            CH = 512
            n_chunks_mlp = math.ceil(SLICE_PAD / CH)

            def mlp_table(t):
                src = emb_sb[min(t, 2)] if t < 3 else emb_sb[0]
                row0 = t * SLICE_PAD if t < 3 else 0
                st = None
                st_subs = 0
                st_row = 0
                for j in range(n_chunks_mlp):
                    c0 = j * CH
                    cw = min(CH, SLICE_PAD - c0)
                    nsub = cw // P
                    ps1 = psA.tile([P, CH], f32, tag="l1", name="ps1")
                    nc.tensor.matmul(
                        ps1[:, :cw], w1_sb[t][:], src[:, c0 : c0 + cw],
                        start=True, stop=True,
                    )
                    u_sb = hpool.tile([P, CH], bf16, tag="u", name="u")
                    nc.scalar.activation(
                        u_sb[:, :cw], ps1[:, :cw],
                        mybir.ActivationFunctionType.Relu, bias=b1_sb[t][:],
                    )
                    # second layer: all nsub sub-matmuls land in ONE [P, CH]
                    # PSUM bank (sequential accumulation groups), then a
                    # single batched DVE bias-add — the per-chunk DVE cost
                    # was the MLP pipeline's rate limiter in the head
                    ps2 = psB.tile([P, CH], f32, tag="l2", name="ps2")
                    for sub in range(nsub):
                        nc.tensor.matmul(
                            ps2[:, sub * P : (sub + 1) * P],
                            u_sb[:, sub * P : (sub + 1) * P], w2s_sb[t][:],
                            start=True, stop=True,
                        )
                    if t < 3:
                        # stage TWO chunks per h_tbl write: SP's per-DMA
                        # issue overhead (~0.8us x 39 writes) was the head's
                        # rate limiter
                        if st is None:
                            st = opool.tile([P, 8, P], bf16, tag="hst", name="hst")
                            st_subs = 0
                            st_row = c0
                        nc.vector.tensor_tensor(
                            out=st[:, st_subs : st_subs + nsub, :],
                            in0=ps2[:, :cw].rearrange("p (s c) -> p s c", s=nsub),
                            in1=b2s_sb[t][:].unsqueeze(1)
                            .broadcast_to([P, nsub, P]),
                            op=mybir.AluOpType.add,
                        )
                        st_subs += nsub
                        if st_subs > 4 or j == n_chunks_mlp - 1:
                            nc.sync.dma_start(
                                out=batched_rows_ap(h_dram, row0 + st_row, st_subs),
                                in_=st[:, :st_subs, :],
                            )
                            st = None
                    else:
                        nc.vector.tensor_tensor(
                            out=self_sb[:, c0 // P : c0 // P + nsub, :],
                            in0=ps2[:, :cw].rearrange("p (s c) -> p s c", s=nsub),
                            in1=b2s_sb[t][:].unsqueeze(1)
                            .broadcast_to([P, nsub, P]),
                            op=mybir.AluOpType.add,
                        )

            for t in range(4):
                mlp_table(t)

            # gidx isn't needed until the first gather prep — issuing its
            # load after the table-write DMAs keeps the head's SP issue
            # queue clear for the h_tbl writes the first gather waits on
            nc.sync.dma_start(out=gidx_sb[:], in_=gidx_p[:, :])

            def debug_out(srct, dt=f32, flat=True):
                for j in range(math.ceil(SLICE / P)):
                    r0 = j * P
                    rw = min(P, SLICE - r0)
                    d_sb = finpool.tile([P, C], dt, tag="dbg", name="dbg")
                    if flat:
                        nc.sync.dma_start(
                            out=d_sb[:],
                            in_=bass.AP(srct, r0 * C, [[C, P], [1, C]]),
                        )
                    else:
                        nc.sync.dma_start(out=d_sb[:], in_=srct[r0 : r0 + P, :])
                    if dt != f32:
                        d32 = finpool.tile([P, C], f32, tag="dbg32", name="dbg32")
                        nc.scalar.copy(d32[:], d_sb[:])
                        d_sb = d32
                    nc.sync.dma_start(out=out_ext[r0 : r0 + rw, :], in_=d_sb[:rw, :])
                t_sb = finpool.tile([1, 1], f32, tag="tok", name="tok2")
                nc.sync.dma_start(out=t_sb[:], in_=tok_p[:, :])
                nc.sync.dma_start(out=tok_out[:, :], in_=t_sb[:])

            run_p2 = debug_phases >= 2
            run_p3 = debug_phases >= 3
            run_p4 = debug_phases >= 4
            if not run_p2:
                tc.strict_bb_all_engine_barrier()
                debug_out(h_dram, bf16, flat=False)

            def issue_rs_chunk(ci):
                ch = chunks[ci]
                L = ch["length"] * P
                n_in = NCORES * C * L
                nc.gpsimd.collective_compute(
                    "ReduceScatter", mybir.AluOpType.add,
                    replica_groups=[list(range(NCORES))],
                    ins=[bass.AP(partial, chunk_base[ci], [[1, n_in]])],
                    outs=[bass.AP(rs_out, ch["rel0"] * C, [[1, C * L]])],
                )

            # Issue each chunk's collective several gather blocks after its
            # last flush is emitted: the Pool SEQ wait (on the chunk's flush
            # DMAs) then resolves ~immediately instead of stalling gather
            # descgen for the next blocks.  The first chunk gets a longer
            # delay — the compute pipeline lags the gather stream most there,
            # and a premature wait starves every engine (~20us in the trace).
            issue_at_tile = {}
            issue_rest = []
            for ci, ch in enumerate(chunks):
                delay = 6 if ci == 0 else 4
                t_issue = (ch["tile_end"] // GBLK + delay) * GBLK
                if t_issue < T_total:
                    issue_at_tile.setdefault(t_issue, []).append(ci)
                else:
                    issue_rest.append(ci)

            def flush_group(q, ps_q):
                # flush WQ=8 consecutive ranks [8q, 8q+8) of channel-major
                # PSUM [ch, 8*128] to the partial blocks; split at
                # (chunk, dest-core) block boundaries (at most 2 pieces)
                f_sb = fpool.tile([P, WQ * P], bf16, tag="f", name="fsb")
                nc.scalar.copy(f_sb[:], ps_q[:])
                r0 = q * WQ
                g0 = 0
                while g0 < WQ:
                    ri = rank_info[r0 + g0]
                    g1 = g0 + 1
                    while (
                        g1 < WQ
                        and rank_info[r0 + g1]["block_base"] == ri["block_base"]
                    ):
                        g1 += 1
                    ncols = (g1 - g0) * P
                    nc.sync.dma_start(
                        out=bass.AP(
                            partial,
                            ri["block_base"] + ri["col0"],
                            [[ri["L"], P], [1, ncols]],
                        ),
                        in_=f_sb[:, g0 * P : g0 * P + ncols],
                    )
                    g0 = g1

            # ---- phase 2: gather + one-hot matmul accumulate (channel-major)
            s_blk = None
            open_ps = {}  # q -> PSUM tile
            gbufs = {}
            state = {"next_b": 0}

            def start_gather_block(b):
                gbuf = gpool.tile([P, GBLK, C], bf16, tag="g", name="g")
                nc.gpsimd.dma_gather(
                    gbuf[:], h_dram.ap(),
                    gidx_sb[:, b * 256 : (b + 1) * 256],
                    GBLK * P, GBLK * P, C, single_packet=False,
                    queue_num=b % 4,
                )
                gbufs[b] = gbuf
                if run_p3 and b * GBLK in issue_at_tile:
                    for ci in issue_at_tile[b * GBLK]:
                        issue_rs_chunk(ci)

            for m_idx, (t, rank, first, last) in enumerate(
                rec_stream if run_p2 else []
            ):
                while t >= state["next_b"] * GBLK and state["next_b"] < B:
                    start_gather_block(state["next_b"])
                    state["next_b"] += 1
                if m_idx % SGRP == 0:
                    n_in_grp = min(SGRP, M_total - m_idx)
                    # transposed batch layout [e, d, grp]: the broadcast
                    # lands on the middle dim so all operands keep a packed
                    # 2-byte last dim -> DVE 2x_1p fast mode
                    s_blk = spool.tile([P, P, SGRP], bf16, tag="s", name="sblk")
                    nc.vector.tensor_tensor(
                        out=s_blk[:, :, :n_in_grp],
                        in0=slots_sb[:, m_idx : m_idx + n_in_grp]
                        .unsqueeze(1).broadcast_to([P, P, n_in_grp]),
                        in1=iota_rep[:, :, :n_in_grp],
                        op=mybir.AluOpType.is_equal,
                    )
                q, wi = rank // WQ, rank % WQ
                if first and q not in open_ps:
                    open_ps[q] = wpool.tile([P, WQ * P], f32, tag="w", name="psq")
                # out[ch, slot] += gbuf^T[ch, e] @ S[e, slot]
                nc.tensor.matmul(
                    open_ps[q][:, wi * P : (wi + 1) * P],
                    gbufs[t // GBLK][:, t % GBLK, :],
                    s_blk[:, :, m_idx % SGRP],
                    start=first, stop=last,
                )
                if last and wi == WQ - 1:
                    flush_group(q, open_ps.pop(q))
            if run_p2:
                while state["next_b"] < B:
                    start_gather_block(state["next_b"])
                    state["next_b"] += 1

            if run_p3:
                for ci in issue_rest:
                    issue_rs_chunk(ci)

            if run_p2 and not run_p3:
                tc.strict_bb_all_engine_barrier()
                debug_out(partial, bf16)
            if run_p3 and not run_p4:
                tc.strict_bb_all_engine_barrier()
                debug_out(rs_out, bf16)

            # ---- phase 3+4: per-chunk finale: transpose back, add self, out
            GF = 8
            for ci in (range(len(chunks)) if run_p4 else ()):
                ch = chunks[ci]
                L = ch["length"] * P
                rs_sb = finpool.tile([P, L], bf16, tag=f"rs{ci}", name=f"rs{ci}")
                nc.sync.dma_start(
                    out=rs_sb[:],
                    in_=bass.AP(rs_out, ch["rel0"] * C, [[L, P], [1, L]]),
                )
                j0 = ch["rel0"] // P
                jend = ch["rel1"] // P
                while j0 < jend and j0 * P < SLICE:
                    g = min(GF, jend - j0)
                    o_sb = finpool.tile([P, GF, P], f32, tag="fo", name="fo")
                    for jj in range(g):
                        jl = j0 + jj - ch["rel0"] // P
                        ps_t = psB.tile([P, P], bf16, tag="l2", name="pst")
                        nc.tensor.transpose(
                            ps_t[:], rs_sb[:, jl * P : (jl + 1) * P], ident_sb[:]
                        )
                        nc.vector.tensor_tensor(
                            out=o_sb[:, jj, :], in0=ps_t[:],
                            in1=self_sb[:, j0 + jj, :], op=mybir.AluOpType.add,
                        )
                    r0 = j0 * P
                    if (j0 + g) * P <= SLICE:
                        nc.sync.dma_start(
                            out=batched_rows_ap(out_ext, r0, g),
                            in_=o_sb[:, :g, :],
                        )
                    else:
                        for jj in range(g):
                            rr = (j0 + jj) * P
                            rw = min(P, SLICE - rr)
                            if rw <= 0:
                                break
                            nc.sync.dma_start(
                                out=out_ext[rr : rr + rw, :],
                                in_=o_sb[:rw, jj, :],
                            )
                    j0 += g

            if run_p4:
                t_sb = finpool.tile([1, 1], f32, tag="tok", name="tokf")
                nc.sync.dma_start(out=t_sb[:], in_=tok_p[:, :])
                nc.sync.dma_start(out=tok_out[:, :], in_=t_sb[:])

    nc.compile()
    return nc


# ---------------------------------------------------------------- entry


def make_in_maps(cfg, inputs):
    """Full problem inputs -> per-core in_maps (+ meta)."""
    N, SLICE, SLICE_PAD = cfg["N"], cfg["SLICE"], cfg["SLICE_PAD"]
    ne = np.asarray(inputs["node_embeddings"], np.float32)
    t = int(inputs["t"])
    assert t == 2 and ne.shape[0] == 3
    ei = np.asarray(inputs["edge_index"])
    ew = np.asarray(inputs["edge_weights"])
    per_core_edges, meta = prep_edges(cfg, ei[0], ei[1], ew)

    # per-hop source embedding layers: hop1 -> ne[t], hop2 -> ne[t-1], hop3 -> ne[t-2]
    layers = [ne[2], ne[1], ne[0]]
    hop_coef = np.asarray(inputs["hop_coef"], np.float32)
    coef4 = np.concatenate([hop_coef, [1.0]]).astype(np.float32)
    iota_in = np.broadcast_to(
        np.arange(P, dtype=np.float32)[None, :], (P, P)
    ).astype(ml_dtypes.bfloat16)
    ident_in = np.eye(P, dtype=np.float32).astype(ml_dtypes.bfloat16)

    w_names = [
        ("rel_W1", "rel_b1", "rel_W2", "rel_b2"),
        None,  # hh index 0
        None,  # hh index 1
        ("loop_W1", "loop_b1", "loop_W2", "loop_b2"),
    ]

    def wset(i):
        if i in (1, 2):
            W1 = np.asarray(inputs["hh_W1"][i - 1], np.float32)
            bb1 = np.asarray(inputs["hh_b1"][i - 1], np.float32)
            W2 = np.asarray(inputs["hh_W2"][i - 1], np.float32)
            bb2 = np.asarray(inputs["hh_b2"][i - 1], np.float32)
        else:
            n1, n2, n3, n4 = w_names[i]
            W1 = np.asarray(inputs[n1], np.float32)
            bb1 = np.asarray(inputs[n2], np.float32)
            W2 = np.asarray(inputs[n3], np.float32)
            bb2 = np.asarray(inputs[n4], np.float32)
        # pre-scale second-layer weights/bias by this hop's coefficient
        return (
            np.ascontiguousarray(W1).astype(ml_dtypes.bfloat16),
            np.ascontiguousarray(bb1[:, None]),
            np.ascontiguousarray(W2 * coef4[i]).astype(ml_dtypes.bfloat16),
            np.broadcast_to((bb2 * coef4[i])[None, :], (P, P)).copy(),
        )

    wsets = [wset(i) for i in range(4)]

    in_maps = []
    for c in range(NCORES):
        m = {}
        for li, name in enumerate(("embA", "embB", "embC")):
            sl = layers[li][c * SLICE : (c + 1) * SLICE]
            pad = np.zeros((P, SLICE_PAD), ml_dtypes.bfloat16)
            pad[:, : sl.shape[0]] = sl.T.astype(ml_dtypes.bfloat16)
            m[name] = pad
        for i in range(4):
            W1, bb1, W2s, bb2s = wsets[i]
            m[f"w1_{i}"] = W1
            m[f"b1_{i}"] = bb1
            m[f"w2s_{i}"] = W2s
            m[f"b2s_{i}"] = bb2s
        m["iota"] = iota_in
        m["ident"] = ident_in
        m["gidx"] = per_core_edges[c]["gidx"]
        m["slots"] = per_core_edges[c]["slots"]
        m["tok"] = np.zeros((1, 1), np.float32)
        in_maps.append(m)
    return in_maps, meta


def kernel(**inputs):
    ei = np.asarray(inputs["edge_index"])
    ne = np.asarray(inputs["node_embeddings"])
    cfg = make_cfg(ne.shape[1], ei.shape[1])
    in_maps, meta = make_in_maps(cfg, inputs)
    nc = build_kernel(cfg, meta)
    res = run_bass_kernel_spmd(nc, in_maps, core_ids=list(range(NCORES)))
    out = np.concatenate([res.results[c]["out"] for c in range(NCORES)], axis=0)
    return out.astype(np.float32)


# revision 40
# speedup vs baseline: 1.0076x; 1.0076x over previous
"""DRew-GIN layer on 8 TRN2 NeuronCores.

Strategy (source-sharded, chunked ReduceScatter, channel-major partials):
  - Nodes are sharded 8 ways. Core c computes the three coef-scaled hop MLP
    tables h'_k = hop_coef[k-1] * MLP_k(emb_src_k) for its node slice only,
    node-major bf16 in local DRAM (W2/b2 are pre-scaled by hop_coef on the
    host; MLP runs bf16 in / f32 PSUM).  The self-loop MLP table for the
    slice stays resident in SBUF.
  - Edges are partitioned by SOURCE core. Each core produces a PARTIAL
    aggregate over the full (padded) destination range: edges are sorted by
    destination window (128 dest rows); per 128-edge tile we dma_gather the
    bf16 source rows (256B each, spread over 4 SWDGE queues) from the local
    h' table, build a one-hot S[e, d] = (slot[e] == d) on DVE (bf16,
    transposed batch layout so the 2x_1p fast mode applies), and
    matmul-accumulate G^T @ S into the window's PSUM tile, which is
    CHANNEL-major [ch, slot] (f32).  Flushes write bf16 channel-major
    blocks with 2KB-per-partition runs (full DMA efficiency).
  - The partial stream is laid out [chunk][dest-core][C][cols]; windows are
    processed in (chunk, dest-core, window) order, so as each chunk's
    windows finish flushing, a contiguous ReduceScatter(add) for just that
    chunk fires, overlapping the collective with later chunks' work.  The
    per-chunk finale PE-transposes back to node-major, adds the
    SBUF-resident self term, and writes the output slice.
"""

import math
import sys

sys.path.insert(0, "/opt/trn_rl_repo")

import ml_dtypes
import numpy as np

import concourse.bacc as bacc
import concourse.bass as bass
import concourse.tile as tile
from concourse import mybir
from concourse.bass_utils import run_bass_kernel_spmd

NCORES = 8
C = 128
P = 128
GBLK = 32  # gather block = 32 tiles = 4096 indices
NCHUNKS = 4  # ReduceScatter pipeline depth


def make_cfg(n_nodes, n_edges):
    assert n_nodes % NCORES == 0
    slice_ = n_nodes // NCORES
    slice_pad = ((slice_ + P - 1) // P) * P
    wps = slice_pad // P  # windows per dest slice
    cfg = dict(
        N=n_nodes,
        E=n_edges,
        SLICE=slice_,
        SLICE_PAD=slice_pad,
        TBL=3 * slice_pad,  # h' table rows per core
        DSLICE_PAD=slice_pad,
        WINDOWS=NCORES * wps,
        WPS=wps,
    )
    return cfg


# ---------------------------------------------------------------- host prep


def chunk_lens(wps, nchunks=NCHUNKS):
    """Uneven chunk split: bigger chunks first, small final chunk so the
    exposed tail (last ReduceScatter + finale) is short."""
    nchunks = min(nchunks, wps)
    base, rem = wps // nchunks, wps % nchunks
    lens = [base + (1 if i < rem else 0) for i in range(nchunks)]
    if UNEVEN and nchunks >= 3 and lens[-1] > 1:
        take = lens[-1] - max(1, lens[-1] // 2)
        lens[-1] -= take
        for i in range(take):
            lens[i % (nchunks - 1)] += 1
    return lens


def prep_edges(cfg, row, col, ew, nchunks=NCHUNKS):
    """Returns (per_core {gidx,slots}, meta).

    Destination windows are ranked in (chunk, dest-core, window-in-chunk)
    order.  The edge stream is packed tightly per (chunk, dest-core) BLOCK
    (padding only at block ends, to the max edge count over source cores);
    within a block, edges sort by (window rank, slot).  Because per-core
    window boundaries fall at different stream positions, each gather tile
    carries one matmul RECORD per window it may span on any core (the
    union range); a record's one-hot column holds a core's slots only for
    its edges of that (tile, window), so off-range cores contribute zeros.
    """
    N, SLICE, SLICE_PAD, WINDOWS, WPS = (
        cfg["N"],
        cfg["SLICE"],
        cfg["SLICE_PAD"],
        cfg["WINDOWS"],
        cfg["WPS"],
    )
    if LENS_OVERRIDE is not None and sum(LENS_OVERRIDE) == WPS:
        lens = list(LENS_OVERRIDE)
    else:
        lens = chunk_lens(WPS, nchunks)
    nchunks = len(lens)
    cumwi = np.concatenate([[0], np.cumsum(lens)])  # wi-space chunk bounds

    # rank order: (chunk, dest core, wi)
    order_w = []
    for ci in range(nchunks):
        for sc in range(NCORES):
            for wi in range(cumwi[ci], cumwi[ci + 1]):
                order_w.append(sc * WPS + wi)
    order_w = np.asarray(order_w)
    wrank = np.empty(WINDOWS, np.int64)
    wrank[order_w] = np.arange(WINDOWS)
    chunk_of_rank = np.repeat(np.arange(nchunks), np.asarray(lens) * NCORES)

    row = row.astype(np.int64)
    col = col.astype(np.int64)
    ew = ew.astype(np.int64)
    s = col // SLICE
    local = col - s * SLICE
    trow = (ew - 1) * SLICE_PAD + local
    assert trow.max() < 3 * SLICE_PAD <= 32767
    w = (row // SLICE) * WPS + (row % SLICE) // P  # dest window
    slot = (row % SLICE) % P
    r = wrank[w]

    key = s * WINDOWS + r
    order = np.argsort(key, kind="stable")
    key_s = key[order]
    counts = np.bincount(key_s, minlength=NCORES * WINDOWS).reshape(NCORES, WINDOWS)

    # blocks: consecutive rank runs of one (chunk, dest core); block bi has
    # ranks [blk_r0[bi], blk_r0[bi+1])
    blk_r0 = [0]
    for ci in range(nchunks):
        for _ in range(NCORES):
            blk_r0.append(blk_r0[-1] + lens[ci])
    nblocks = len(blk_r0) - 1

    # per-core positions: edges sorted by (core, rank); position within the
    # (core, BLOCK) group = cumulative count within the block
    group_starts = np.concatenate([[0], np.cumsum(counts.reshape(-1))])[:-1]
    pos_in_rank = np.arange(len(key_s)) - group_starts[key_s]
    core_of = key_s // WINDOWS
    rank_of = key_s % WINDOWS

    # per-core cumulative start of each rank within its block
    rank_block = np.zeros(WINDOWS, np.int64)  # rank -> block
    for bi in range(nblocks):
        rank_block[blk_r0[bi] : blk_r0[bi + 1]] = bi
    start_in_block = np.zeros((NCORES, WINDOWS), np.int64)
    for bi in range(nblocks):
        rr = slice(blk_r0[bi], blk_r0[bi + 1])
        c = counts[:, rr]
        start_in_block[:, rr] = np.cumsum(c, axis=1) - c

    # block tile counts (max over cores) and offsets
    blk_tiles = np.zeros(nblocks, np.int64)
    for bi in range(nblocks):
        tot = counts[:, blk_r0[bi] : blk_r0[bi + 1]].sum(axis=1)
        blk_tiles[bi] = max(1, int(np.ceil(tot.max() / P)))
    blk_t0 = np.concatenate([[0], np.cumsum(blk_tiles)])
    T_total = int(blk_t0[-1])
    B = (T_total + GBLK - 1) // GBLK

    # matmul records: per window, union tile range over cores (global tiles)
    u0 = np.zeros(WINDOWS, np.int64)
    u1 = np.zeros(WINDOWS, np.int64)
    for rr in range(WINDOWS):
        bi = rank_block[rr]
        st = start_in_block[:, rr]
        en = st + counts[:, rr]
        nz = counts[:, rr] > 0
        if nz.any():
            t0 = (st[nz] // P).min()
            t1 = int(np.ceil(en[nz] / P).max())
        else:
            t0 = int(st.min() // P)
            t1 = t0 + 1
        u0[rr] = blk_t0[bi] + t0
        u1[rr] = blk_t0[bi] + max(t1, t0 + 1)

    # records in WINDOW-sequential order (PSUM start=True zeroes a whole
    # 2KB bank region, so windows sharing a bank must accumulate strictly
    # sequentially); tile index regresses by at most span-1 between
    # windows, staying within the live gather blocks
    rec_stream = []
    rec_col = {}
    for bi in range(nblocks):
        for rr in range(blk_r0[bi], blk_r0[bi + 1]):
            for t in range(int(u0[rr]), int(u1[rr])):
                rec_col[(t, rr)] = len(rec_stream)
                rec_stream.append(
                    (int(t), int(rr), bool(t == u0[rr]), bool(t == u1[rr] - 1))
                )
    M_total = len(rec_stream)

    # chunk metadata
    chunks = []
    for ci in range(nchunks):
        rank1 = int(cumwi[ci + 1]) * NCORES
        tile_end = int(blk_t0[np.searchsorted(blk_r0, rank1)]) - 1
        chunks.append(
            dict(
                rank0=int(cumwi[ci]) * NCORES,
                rank1=rank1,
                tile_end=tile_end,
                rel0=int(cumwi[ci]) * P,
                rel1=int(cumwi[ci + 1]) * P,
                length=int(lens[ci]),
            )
        )

    # per-core streams
    NPAD = T_total * P
    trow_s = trow[order]
    slot_s = slot[order]
    # global stream position of each edge: block tile base + in-block pos
    pos_block = start_in_block[core_of, rank_of] + pos_in_rank
    stream_pos = blk_t0[rank_block[rank_of]] * P + pos_block
    edge_tile = stream_pos // P
    edge_lane = stream_pos % P
    edge_colkey = [rec_col[(int(t), int(rr))] for t, rr in zip(edge_tile, rank_of)]
    edge_colkey = np.asarray(edge_colkey, np.int64)

    per_core = []
    for c in range(NCORES):
        m = core_of == c
        gidx = np.zeros(NPAD, np.int16)  # dummy -> row 0
        gidx[stream_pos[m]] = trow_s[m].astype(np.int16)
        slots = np.full((M_total, P), 255, np.float32)
        slots[edge_colkey[m], edge_lane[m]] = slot_s[m].astype(np.float32)
        # wrap gidx for dma_gather: block b, idx j -> [j%16, b*256 + j//16]
        blk = np.zeros(B * GBLK * P, np.int16)
        blk[:NPAD] = gidx
        blk = blk.reshape(B, GBLK * P // 16, 16).transpose(0, 2, 1)  # [B,16,256]
        blk = np.tile(blk, (1, 8, 1))  # [B,128,256]
        gidx_in = blk.transpose(1, 0, 2).reshape(P, -1).copy()  # [128, B*256]
        slots_in = slots.T.astype(ml_dtypes.bfloat16)  # [128, M_total]
        per_core.append(dict(gidx=gidx_in, slots=slots_in))
    meta = dict(
        rec_stream=rec_stream,
        T_total=T_total,
        M_total=M_total,
        B=B,
        chunks=chunks,
    )
    return per_core, meta


# ---------------------------------------------------------------- builder


def build_kernel(cfg, meta, debug_phases=4, comm_mode="rsc"):
    assert comm_mode == "rsc"
    SLICE, SLICE_PAD, TBL, DSLICE_PAD, WINDOWS = (
        cfg["SLICE"],
        cfg["SLICE_PAD"],
        cfg["TBL"],
        cfg["DSLICE_PAD"],
        cfg["WINDOWS"],
    )
    T_total, B = meta["T_total"], meta["B"]
    M_total = meta["M_total"]
    rec_stream = meta["rec_stream"]
    chunks = meta["chunks"]
    PTOT = NCORES * DSLICE_PAD
    f32 = mybir.dt.float32
    bf16 = mybir.dt.bfloat16
    SGRP = 8          # one-hot compare batch (tiles per DVE op)
    WQ = 8            # windows per PSUM flush group
    assert WINDOWS % WQ == 0

    # rank -> (chunk, dest core, col0) for the channel-major partial layout:
    # partial = [chunk][dest core][C rows][len_c*128 cols], flattened
    rank_info = []
    chunk_base = []
    off = 0
    for ci, ch in enumerate(chunks):
        chunk_base.append(off)
        L = ch["length"] * P  # cols per (chunk, dest-core) block
        for rr in range(ch["rank0"], ch["rank1"]):
            rl = rr - ch["rank0"]
            s_dst, wi_l = rl // ch["length"], rl % ch["length"]
            rank_info.append(
                dict(chunk=ci, block_base=off + s_dst * C * L, col0=wi_l * P, L=L)
            )
        off += NCORES * C * L
    assert off == PTOT * C

    nc = bacc.Bacc(num_swdge_queues=4)

    def param(name, shape, dt=f32):
        return nc.declare_dram_parameter(name, list(shape), dt, isOutput=False)

    embp = [param(n, [P, SLICE_PAD], bf16) for n in ("embA", "embB", "embC")]
    w1 = [param(f"w1_{i}", [P, P], bf16) for i in range(4)]  # rel, hh0, hh1, loop
    b1 = [param(f"b1_{i}", [P, 1]) for i in range(4)]
    # W2/b2 pre-scaled by hop_coef on the host
    w2s = [param(f"w2s_{i}", [P, P], bf16) for i in range(4)]
    b2s = [param(f"b2s_{i}", [P, P]) for i in range(4)]  # row-broadcast
    iota_p = param("iota", [P, P], bf16)
    ident_p = param("ident", [P, P], bf16)
    gidx_p = param("gidx", [P, B * 256], mybir.dt.int16)
    slots_p = param("slots", [P, M_total], bf16)
    tok_p = param("tok", [1, 1])
    out_ext = nc.declare_dram_parameter("out", [SLICE, C], f32, isOutput=True)
    tok_out = nc.declare_dram_parameter("tok_out", [1, 1], f32, isOutput=True)

    h_dram = nc.dram_tensor("h_tbl", [TBL, C], bf16)
    partial = nc.dram_tensor("partial", [PTOT * C], bf16)  # channel-major blocks
    rs_out = nc.dram_tensor("rs_out", [DSLICE_PAD * C], bf16)  # ch-major blocks

    def batched_rows_ap(handle, r0, nsub):
        # [p, s, ch] view of rows [r0, r0 + nsub*128) of a [rows, C] tensor
        return bass.AP(handle, r0 * C, [[C, P], [P * C, nsub], [1, C]])

    with tile.TileContext(nc) as tc:
        with (
            tc.tile_pool(name="resident", bufs=1) as rpool,
            tc.tile_pool(name="hid", bufs=3) as hpool,
            tc.tile_pool(name="hstage", bufs=4) as opool,
            tc.tile_pool(name="gather", bufs=6) as gpool,
            tc.tile_pool(name="onehot", bufs=3) as spool,
            tc.tile_pool(name="flush", bufs=4) as fpool,
            tc.tile_pool(name="fin", bufs=2) as finpool,
            tc.tile_pool(name="psA", bufs=2, space="PSUM") as psA,
            tc.tile_pool(name="psB", bufs=2, space="PSUM") as psB,
            tc.tile_pool(name="win", bufs=2, space="PSUM") as wpool,
        ):
            # ---- phase 0: resident loads
            w1_sb = [rpool.tile([P, P], bf16, tag=f"w1_{i}", name=f"w1sb{i}") for i in range(4)]
            b1_sb = [rpool.tile([P, 1], f32, tag=f"b1_{i}", name=f"b1sb{i}") for i in range(4)]
            w2s_sb = [rpool.tile([P, P], bf16, tag=f"w2s_{i}", name=f"w2ssb{i}") for i in range(4)]
            b2s_sb = [rpool.tile([P, P], f32, tag=f"b2s_{i}", name=f"b2ssb{i}") for i in range(4)]
            emb_sb = [rpool.tile([P, SLICE_PAD], bf16, tag=f"emb_{i}", name=f"embsb{i}") for i in range(3)]
            iota_sb = rpool.tile([P, P], bf16, tag="iota")
            ident_sb = rpool.tile([P, P], bf16, tag="ident")
            iota_rep = rpool.tile([P, P, SGRP], bf16, tag="iotarep", name="iotarep")
            gidx_sb = rpool.tile([P, B * 256], mybir.dt.int16, tag="gidx")
            slots_sb = rpool.tile([P, M_total], bf16, tag="slots")

            # embeddings first on SP (they gate the first MLP matmul and the
            # h_tbl writes the first gather waits on); weights and one-hot
            # metadata go through the Activation engine's HWDGE queue, which
            # is idle in the head, so SP's issue queue stays clear
            for i in range(3):
                nc.sync.dma_start(out=emb_sb[i][:], in_=embp[i][:, :])
            for i in range(4):
                nc.scalar.dma_start(out=w1_sb[i][:], in_=w1[i][:, :])
                nc.scalar.dma_start(out=b1_sb[i][:], in_=b1[i][:, :])
                nc.scalar.dma_start(out=w2s_sb[i][:], in_=w2s[i][:, :])
                nc.scalar.dma_start(out=b2s_sb[i][:], in_=b2s[i][:, :])
            nc.scalar.dma_start(out=iota_sb[:], in_=iota_p[:, :])
            nc.scalar.dma_start(out=ident_sb[:], in_=ident_p[:, :])
            # slots loads early: the DVE one-hot prebuilds consume it during
            # the table phase's DVE slack
            nc.scalar.dma_start(out=slots_sb[:], in_=slots_p[:, :])
            # iota replicated along the group dim: real packed tile so the
            # one-hot is_equal qualifies for the DVE 2x_1p fast mode
            nc.vector.tensor_copy(
                iota_rep[:], iota_sb[:].unsqueeze(2).broadcast_to([P, P, SGRP])
            )

            # ---- phase 1: MLP tables (hops 1..3 from embA/B/C; self from embA)
            # self table stays resident in SBUF (read directly by the finale)
            n_sub_tot = SLICE_PAD // P
            self_sb = rpool.tile([P, n_sub_tot, P], f32, tag="selft", name="selft")
            CH = 512
            n_chunks_mlp = math.ceil(SLICE_PAD / CH)

            def mlp_table(t):
                src = emb_sb[min(t, 2)] if t < 3 else emb_sb[0]
                row0 = t * SLICE_PAD if t < 3 else 0
                st = None
                st_subs = 0
                st_row = 0
                for j in range(n_chunks_mlp):
                    c0 = j * CH
                    cw = min(CH, SLICE_PAD - c0)
                    nsub = cw // P
                    ps1 = psA.tile([P, CH], f32, tag="l1", name="ps1")
                    nc.tensor.matmul(
                        ps1[:, :cw], w1_sb[t][:], src[:, c0 : c0 + cw],
                        start=True, stop=True,
                    )
                    u_sb = hpool.tile([P, CH], bf16, tag="u", name="u")
                    nc.scalar.activation(
                        u_sb[:, :cw], ps1[:, :cw],
                        mybir.ActivationFunctionType.Relu, bias=b1_sb[t][:],
                    )
                    # second layer: all nsub sub-matmuls land in ONE [P, CH]
                    # PSUM bank (sequential accumulation groups), then a
                    # single batched DVE bias-add — the per-chunk DVE cost
                    # was the MLP pipeline's rate limiter in the head
                    ps2 = psB.tile([P, CH], f32, tag="l2", name="ps2")
                    for sub in range(nsub):
                        nc.tensor.matmul(
                            ps2[:, sub * P : (sub + 1) * P],
                            u_sb[:, sub * P : (sub + 1) * P], w2s_sb[t][:],
                            start=True, stop=True,
                        )
                    if t < 3:
                        # stage TWO chunks per h_tbl write: SP's per-DMA
                        # issue overhead (~0.8us x 39 writes) was the head's
                        # rate limiter
                        if st is None:
                            st = opool.tile([P, 8, P], bf16, tag="hst", name="hst")
                            st_subs = 0
                            st_row = c0
                        nc.vector.tensor_tensor(
                            out=st[:, st_subs : st_subs + nsub, :],
                            in0=ps2[:, :cw].rearrange("p (s c) -> p s c", s=nsub),
                            in1=b2s_sb[t][:].unsqueeze(1)
                            .broadcast_to([P, nsub, P]),
                            op=mybir.AluOpType.add,
                        )
                        st_subs += nsub
                        if st_subs > 4 or j == n_chunks_mlp - 1:
                            nc.sync.dma_start(
                                out=batched_rows_ap(h_dram, row0 + st_row, st_subs),
                                in_=st[:, :st_subs, :],
                            )
                            st = None
                    else:
                        nc.vector.tensor_tensor(
                            out=self_sb[:, c0 // P : c0 // P + nsub, :],
                            in0=ps2[:, :cw].rearrange("p (s c) -> p s c", s=nsub),
                            in1=b2s_sb[t][:].unsqueeze(1)
                            .broadcast_to([P, nsub, P]),
                            op=mybir.AluOpType.add,
                        )

            for t in range(4):
                mlp_table(t)

            # gidx isn't needed until the first gather prep — issuing its
            # load after the table-write DMAs keeps the head's SP issue
            # queue clear for the h_tbl writes the first gather waits on
            nc.sync.dma_start(out=gidx_sb[:], in_=gidx_p[:, :])

            def debug_out(srct, dt=f32, flat=True):
                for j in range(math.ceil(SLICE / P)):
                    r0 = j * P
                    rw = min(P, SLICE - r0)
                    d_sb = finpool.tile([P, C], dt, tag="dbg", name="dbg")
                    if flat:
                        nc.sync.dma_start(
                            out=d_sb[:],
                            in_=bass.AP(srct, r0 * C, [[C, P], [1, C]]),
                        )
                    else:
                        nc.sync.dma_start(out=d_sb[:], in_=srct[r0 : r0 + P, :])
                    if dt != f32:
                        d32 = finpool.tile([P, C], f32, tag="dbg32", name="dbg32")
                        nc.scalar.copy(d32[:], d_sb[:])
                        d_sb = d32
                    nc.sync.dma_start(out=out_ext[r0 : r0 + rw, :], in_=d_sb[:rw, :])
                t_sb = finpool.tile([1, 1], f32, tag="tok", name="tok2")
                nc.sync.dma_start(out=t_sb[:], in_=tok_p[:, :])
                nc.sync.dma_start(out=tok_out[:, :], in_=t_sb[:])

            run_p2 = debug_phases >= 2
            run_p3 = debug_phases >= 3
            run_p4 = debug_phases >= 4
            if not run_p2:
                tc.strict_bb_all_engine_barrier()
                debug_out(h_dram, bf16, flat=False)

            def issue_rs_chunk(ci):
                ch = chunks[ci]
                L = ch["length"] * P
                n_in = NCORES * C * L
                nc.gpsimd.collective_compute(
                    "ReduceScatter", mybir.AluOpType.add,
                    replica_groups=[list(range(NCORES))],
                    ins=[bass.AP(partial, chunk_base[ci], [[1, n_in]])],
                    outs=[bass.AP(rs_out, ch["rel0"] * C, [[1, C * L]])],
                )

            # Issue each chunk's collective several gather blocks after its
            # last flush is emitted: the Pool SEQ wait (on the chunk's flush
            # DMAs) then resolves ~immediately instead of stalling gather
            # descgen for the next blocks.  The first chunk gets a longer
            # delay — the compute pipeline lags the gather stream most there,
            # and a premature wait starves every engine (~20us in the trace).
            issue_at_tile = {}
            issue_rest = []
            for ci, ch in enumerate(chunks):
                delay = 6 if ci == 0 else 4
                t_issue = (ch["tile_end"] // GBLK + delay) * GBLK
                if t_issue < T_total:
                    issue_at_tile.setdefault(t_issue, []).append(ci)
                else:
                    issue_rest.append(ci)

            def flush_group(q, ps_q):
                # flush WQ=8 consecutive ranks [8q, 8q+8) of channel-major
                # PSUM [ch, 8*128] to the partial blocks; split at
                # (chunk, dest-core) block boundaries (at most 2 pieces)
                f_sb = fpool.tile([P, WQ * P], bf16, tag="f", name="fsb")
                nc.scalar.copy(f_sb[:], ps_q[:])
                r0 = q * WQ
                g0 = 0
                while g0 < WQ:
                    ri = rank_info[r0 + g0]
                    g1 = g0 + 1
                    while (
                        g1 < WQ
                        and rank_info[r0 + g1]["block_base"] == ri["block_base"]
                    ):
                        g1 += 1
                    ncols = (g1 - g0) * P
                    nc.sync.dma_start(
                        out=bass.AP(
                            partial,
                            ri["block_base"] + ri["col0"],
                            [[ri["L"], P], [1, ncols]],
                        ),
                        in_=f_sb[:, g0 * P : g0 * P + ncols],
                    )
                    g0 = g1

            # ---- phase 2: gather + one-hot matmul accumulate (channel-major)
            s_blk = None
            open_ps = {}  # q -> PSUM tile
            gbufs = {}
            state = {"next_b": 0}

            def start_gather_block(b):
                gbuf = gpool.tile([P, GBLK, C], bf16, tag="g", name="g")
                nc.gpsimd.dma_gather(
                    gbuf[:], h_dram.ap(),
                    gidx_sb[:, b * 256 : (b + 1) * 256],
                    GBLK * P, GBLK * P, C, single_packet=False,
                    queue_num=b % 4,
                )
                gbufs[b] = gbuf
                if run_p3 and b * GBLK in issue_at_tile:
                    for ci in issue_at_tile[b * GBLK]:
                        issue_rs_chunk(ci)

            for m_idx, (t, rank, first, last) in enumerate(
                rec_stream if run_p2 else []
            ):
                while t >= state["next_b"] * GBLK and state["next_b"] < B:
                    start_gather_block(state["next_b"])
                    state["next_b"] += 1
                if m_idx % SGRP == 0:
                    n_in_grp = min(SGRP, M_total - m_idx)
                    # transposed batch layout [e, d, grp]: the broadcast
                    # lands on the middle dim so all operands keep a packed
                    # 2-byte last dim -> DVE 2x_1p fast mode
                    s_blk = spool.tile([P, P, SGRP], bf16, tag="s", name="sblk")
                    nc.vector.tensor_tensor(
                        out=s_blk[:, :, :n_in_grp],
                        in0=slots_sb[:, m_idx : m_idx + n_in_grp]
                        .unsqueeze(1).broadcast_to([P, P, n_in_grp]),
                        in1=iota_rep[:, :, :n_in_grp],
                        op=mybir.AluOpType.is_equal,
                    )
                q, wi = rank // WQ, rank % WQ
                if first and q not in open_ps:
                    open_ps[q] = wpool.tile([P, WQ * P], f32, tag="w", name="psq")
                # out[ch, slot] += gbuf^T[ch, e] @ S[e, slot]
                nc.tensor.matmul(
                    open_ps[q][:, wi * P : (wi + 1) * P],
                    gbufs[t // GBLK][:, t % GBLK, :],
                    s_blk[:, :, m_idx % SGRP],
                    start=first, stop=last,
                )
                if last and wi == WQ - 1:
                    flush_group(q, open_ps.pop(q))
            if run_p2:
                while state["next_b"] < B:
                    start_gather_block(state["next_b"])
                    state["next_b"] += 1

            if run_p3:
                for ci in issue_rest:
                    issue_rs_chunk(ci)

            if run_p2 and not run_p3:
                tc.strict_bb_all_engine_barrier()
                debug_out(partial, bf16)
            if run_p3 and not run_p4:
                tc.strict_bb_all_engine_barrier()
                debug_out(rs_out, bf16)

            # ---- phase 3+4: per-chunk finale: transpose back, add self, out
            GF = 8
            for ci in (range(len(chunks)) if run_p4 else ()):
                ch = chunks[ci]
                L = ch["length"] * P
                rs_sb = finpool.tile([P, L], bf16, tag=f"rs{ci}", name=f"rs{ci}")
                nc.sync.dma_start(
                    out=rs_sb[:],
                    in_=bass.AP(rs_out, ch["rel0"] * C, [[L, P], [1, L]]),
                )
                j0 = ch["rel0"] // P
                jend = ch["rel1"] // P
                while j0 < jend and j0 * P < SLICE:
                    g = min(GF, jend - j0)
                    o_sb = finpool.tile([P, GF, P], f32, tag="fo", name="fo")
                    for jj in range(g):
                        jl = j0 + jj - ch["rel0"] // P
                        ps_t = psB.tile([P, P], bf16, tag="l2", name="pst")
                        nc.tensor.transpose(
                            ps_t[:], rs_sb[:, jl * P : (jl + 1) * P], ident_sb[:]
                        )
                        nc.vector.tensor_tensor(
                            out=o_sb[:, jj, :], in0=ps_t[:],
                            in1=self_sb[:, j0 + jj, :], op=mybir.AluOpType.add,
                        )
                    r0 = j0 * P
                    if (j0 + g) * P <= SLICE:
                        nc.sync.dma_start(
                            out=batched_rows_ap(out_ext, r0, g),
                            in_=o_sb[:, :g, :],
                        )
                    else:
                        for jj in range(g):
                            rr = (j0 + jj) * P
                            rw = min(P, SLICE - rr)
                            if rw <= 0:
                                break
                            nc.sync.dma_start(
                                out=out_ext[rr : rr + rw, :],
                                in_=o_sb[:rw, jj, :],
                            )
                    j0 += g

            if run_p4:
                t_sb = finpool.tile([1, 1], f32, tag="tok", name="tokf")
                nc.sync.dma_start(out=t_sb[:], in_=tok_p[:, :])
                nc.sync.dma_start(out=tok_out[:, :], in_=t_sb[:])

    nc.compile()
    return nc


# ---------------------------------------------------------------- entry


def make_in_maps(cfg, inputs):
    """Full problem inputs -> per-core in_maps (+ meta)."""
    N, SLICE, SLICE_PAD = cfg["N"], cfg["SLICE"], cfg["SLICE_PAD"]
    ne = np.asarray(inputs["node_embeddings"], np.float32)
    t = int(inputs["t"])
    assert t == 2 and ne.shape[0] == 3
    ei = np.asarray(inputs["edge_index"])
    ew = np.asarray(inputs["edge_weights"])
    per_core_edges, meta = prep_edges(cfg, ei[0], ei[1], ew)

    # per-hop source embedding layers: hop1 -> ne[t], hop2 -> ne[t-1], hop3 -> ne[t-2]
    layers = [ne[2], ne[1], ne[0]]
    hop_coef = np.asarray(inputs["hop_coef"], np.float32)
    coef4 = np.concatenate([hop_coef, [1.0]]).astype(np.float32)
    iota_in = np.broadcast_to(
        np.arange(P, dtype=np.float32)[None, :], (P, P)
    ).astype(ml_dtypes.bfloat16)
    ident_in = np.eye(P, dtype=np.float32).astype(ml_dtypes.bfloat16)

    w_names = [
        ("rel_W1", "rel_b1", "rel_W2", "rel_b2"),
        None,  # hh index 0
        None,  # hh index 1
        ("loop_W1", "loop_b1", "loop_W2", "loop_b2"),
    ]

    def wset(i):
        if i in (1, 2):
            W1 = np.asarray(inputs["hh_W1"][i - 1], np.float32)
            bb1 = np.asarray(inputs["hh_b1"][i - 1], np.float32)
            W2 = np.asarray(inputs["hh_W2"][i - 1], np.float32)
            bb2 = np.asarray(inputs["hh_b2"][i - 1], np.float32)
        else:
            n1, n2, n3, n4 = w_names[i]
            W1 = np.asarray(inputs[n1], np.float32)
            bb1 = np.asarray(inputs[n2], np.float32)
            W2 = np.asarray(inputs[n3], np.float32)
            bb2 = np.asarray(inputs[n4], np.float32)
        # pre-scale second-layer weights/bias by this hop's coefficient
        return (
            np.ascontiguousarray(W1).astype(ml_dtypes.bfloat16),
            np.ascontiguousarray(bb1[:, None]),
            np.ascontiguousarray(W2 * coef4[i]).astype(ml_dtypes.bfloat16),
            np.broadcast_to((bb2 * coef4[i])[None, :], (P, P)).copy(),
        )

    wsets = [wset(i) for i in range(4)]

    in_maps = []
    for c in range(NCORES):
        m = {}
        for li, name in enumerate(("embA", "embB", "embC")):
            sl = layers[li][c * SLICE : (c + 1) * SLICE]
            pad = np.zeros((P, SLICE_PAD), ml_dtypes.bfloat16)
            pad[:, : sl.shape[0]] = sl.T.astype(ml_dtypes.bfloat16)
            m[name] = pad
        for i in range(4):
            W1, bb1, W2s, bb2s = wsets[i]
            m[f"w1_{i}"] = W1
            m[f"b1_{i}"] = bb1
            m[f"w2s_{i}"] = W2s
            m[f"b2s_{i}"] = bb2s
        m["iota"] = iota_in
        m["ident"] = ident_in
        m["gidx"] = per_core_edges[c]["gidx"]
        m["slots"] = per_core_edges[c]["slots"]
        m["tok"] = np.zeros((1, 1), np.float32)
        in_maps.append(m)
    return in_maps, meta


def kernel(**inputs):
    ei = np.asarray(inputs["edge_index"])
    ne = np.asarray(inputs["node_embeddings"])
    cfg = make_cfg(ne.shape[1], ei.shape[1])
    in_maps, meta = make_in_maps(cfg, inputs)
    nc = build_kernel(cfg, meta)
    res = run_bass_kernel_spmd(nc, in_maps, core_ids=list(range(NCORES)))
    out = np.concatenate([res.results[c]["out"] for c in range(NCORES)], axis=0)
    return out.astype(np.float32)
